# revision 41
# baseline (speedup 1.0000x reference)
"""Distributed HGT message-passing kernel for 8 Trainium2 NeuronCores.

Wire-optimized dst-sharded design (host->device transfer dominates
end-to-end time, so inputs are kept minimal and everything derivable is
built on device):
  - Each core ships three packed blobs: a bf16 blob (its own dst shard
    of x + dst-local ids + iota const), an int16 blob (unreplicated
    gather indices, replicated across partitions on device), and a 1/8
    shard of the replicated weights (bf16).
  - On-device AllGathers assemble the full node-feature table and the
    full weight set; each core then gathers the unique boundary source
    features its edges reference (transposed dma_gather, per-owner
    bucket bases so indices fit int16) and builds compact
    kv_r = [k_raw || v @ mrel_r] tables.
  - One-hot dst masks for the edge phase are built on device from the
    dst-local ids (iota compare + PE transpose), not shipped.
  - The relation key-transform is folded into the query side
    (q~ = q @ arel^T * prel/sqrt(D)); softmax skips max-subtraction
    (logits are O(1)); exp runs in fp32.  Skip path and final output
    are bf16 (host upcasts).
"""

import math
import sys
from contextlib import ExitStack

import numpy as np
import ml_dtypes

sys.path.insert(0, "/opt/trn_rl_repo")

# Persistent XLA compilation cache: run_bass_kernel_spmd re-jits a fresh
# closure every call, which otherwise re-runs the whole backend compile
# (incl. the BIR->NEFF hook) each time.  The disk cache makes repeat calls
# hit a deserialize instead.
import os as _os  # noqa: E402
import jax as _jax  # noqa: E402

_os.makedirs("/tmp/jax_bass_cache", exist_ok=True)
_jax.config.update("jax_compilation_cache_dir", "/tmp/jax_bass_cache")
_jax.config.update("jax_persistent_cache_min_compile_time_secs", 0)
_jax.config.update("jax_persistent_cache_min_entry_size_bytes", 0)

import concourse.bass as bass  # noqa: E402,F401
import concourse.mybir as mybir  # noqa: E402
import concourse.tile as tile  # noqa: E402
from concourse import bacc  # noqa: E402
from concourse.bass_utils import run_bass_kernel_spmd  # noqa: E402
from concourse.masks import make_identity  # noqa: E402

BF16 = ml_dtypes.bfloat16
N, E, C, H, D = 50000, 200000, 512, 8, 64
NCORES = 8
NSH = N // NCORES          # 6250 dst nodes per type per core
P = 128
NBLK = (NSH + P - 1) // P  # 49 dst blocks
NSHP = NBLK * P            # 6272 padded
XROWS = 2 * NSHP           # 12544 rows per core in the x shard (A then B)
XFULL = NCORES * XROWS     # 100352 rows after AllGather
TB = 5                     # edge tiles per dst block (640 edge slots)
NTILES = NBLK * TB         # 245 tiles per relation per core
UCB = 3072                 # compact-table rows per owner bucket (6 x 512)
UCHUNK = UCB // 512        # gather chunks per bucket
UCAP = NCORES * UCB        # 24576 compact rows per relation (< int16 max)
RELS = [("r1", "B", "A"), ("r2", "A", "B"), ("r3", "A", "A")]
TYPE_RELS = [("B", ["r1"]), ("A", ["r2", "r3"])]
OUT_S = 3.5                 # 12-bit output scale: q = round(fo * 2047/OUT_S)
OUT_MAGIC = 12582912.0      # 1.5 * 2**23 -- f32 add forces RNE to integer

f32 = mybir.dt.float32
bf = mybir.dt.bfloat16
i16 = mybir.dt.int16
AF = mybir.ActivationFunctionType
OP = mybir.AluOpType

# ---- packed-blob layouts (element offsets, all static) ----
W_MANIFEST = [
    ("kW_A", (4, P, C)), ("kW_B", (4, P, C)),
    ("qWT_A", (8, D, C)), ("qWT_B", (8, D, C)),
    ("vWT_A", (8, D, C)), ("vWT_B", (8, D, C)),
    ("oW_A", (4, P, C)), ("oW_B", (4, P, C)),
    ("linW", (4, P, 128)),
    ("mrel_r1", (D, C)), ("arelT_r1", (D, C)),
    ("mrel_r2", (D, C)), ("arelT_r2", (D, C)),
    ("mrel_r3", (D, C)), ("arelT_r3", (D, C)),
]
W_OFF = {}
_o = 0
for _nm, _sh in W_MANIFEST:
    W_OFF[_nm] = _o
    _o += int(np.prod(_sh))
W_TOTAL = _o
WS = -(-W_TOTAL // NCORES)          # per-core weight-shard elements
W_PAD = WS * NCORES

X_DL = {r: i * P * NTILES for i, (r, _, _) in enumerate(RELS)}
X_IOTA = 3 * P * NTILES
XB = X_IOTA + P * TB * P
# packed e6m5 x: hi-byte plane + low-nibble-pair plane (1.5 B/elem)
Q8_HI = XROWS * C
Q8B = Q8_HI + XROWS * C // 2
XCHUNK = 256                        # decoded rows per decode tile
NXCH = XROWS // XCHUNK              # 49

I_UID = {r: i * (UCAP // 16) * 16 for i, (r, _, _) in enumerate(RELS)}
_ib = 3 * UCAP
I_EID = {r: _ib + i * NTILES * 8 * 16 for i, (r, _, _) in enumerate(RELS)}
IB = _ib + 3 * NTILES * P


# ---------------------------------------------------------------------------
# Host-side preprocessing (index routing + layout staging only)
# ---------------------------------------------------------------------------

def _wrap16(flat):
    """[n] -> [16, n//16] gather-index layout (16-partition wrap)."""
    return np.ascontiguousarray(flat.reshape(-1, 16).T)


def _prep_core(core, inp):
    lo = core * NSH
    xb = np.zeros(XB, BF16)
    xb[X_IOTA:X_IOTA + P * TB * P] = np.tile(
        np.arange(P, dtype=np.float32).astype(BF16), P * TB)

    # pack x shard as e6m5: v = s<<11 | E6<<5 | M5  (E6 = bf16 exp - 96)
    xrows = np.zeros((XROWS, C), np.float32)
    xrows[:NSH] = inp["x_A"][lo:lo + NSH]
    xrows[NSHP:NSHP + NSH] = inp["x_B"][lo:lo + NSH]
    u = xrows.astype(BF16).view(np.uint16).astype(np.uint32)
    r = (u + 2) & 0xFFFC                      # round, drop 2 mantissa bits
    s = (r >> 15) & 1
    ebf = (r >> 7) & 0xFF
    m5 = (r >> 2) & 0x1F
    e6 = ebf.astype(np.int64) - 96
    v = np.where(e6 < 0, 0,
                 (s << 11) | (np.maximum(e6, 0) << 5) | m5).astype(np.uint16)
    q8 = np.zeros(Q8B, np.uint8)
    q8[:Q8_HI] = (v >> 4).astype(np.uint8).ravel()
    vf = v.ravel()
    q8[Q8_HI:] = (((vf[0::2] & 0xF) << 4) | (vf[1::2] & 0xF)).astype(np.uint8)

    ib = np.zeros(IB, np.int16)
    for r, T, S in RELS:
        ei = inp[f"ei_{r}"]
        src, dst = ei[0], ei[1]
        sel = (dst >= lo) & (dst < lo + NSH)
        src, dst = src[sel], dst[sel] - lo

        usrc, pos = np.unique(src, return_inverse=True)
        owner = usrc // NSH
        bcnt = np.bincount(owner, minlength=NCORES)
        assert bcnt.max() <= UCB, bcnt.max()
        boff = np.zeros(NCORES + 1, np.int64)
        boff[1:] = np.cumsum(bcnt)
        crow = owner * UCB + (np.arange(len(usrc)) - boff[owner])
        ulocal = np.zeros(UCAP, np.int64)
        ulocal[crow] = usrc % NSH
        ib[I_UID[r]:I_UID[r] + UCAP] = _wrap16(ulocal.astype(np.int16)).ravel()

        blk = dst // P
        cnt = np.bincount(blk, minlength=NBLK)
        assert cnt.max() <= TB * P, cnt.max()
        order = np.argsort(blk, kind="stable")
        epos = crow[pos][order]
        dloc = (dst[order] % P)

        eidx_flat = np.zeros(NTILES * P, np.int16)
        dl_flat = np.full(NTILES * P, 200.0, np.float64)  # 200 => empty slot
        off = 0
        for b in range(NBLK):
            nb_e = cnt[b]
            base = b * TB * P
            eidx_flat[base:base + nb_e] = epos[off:off + nb_e]
            dl_flat[base:base + nb_e] = dloc[off:off + nb_e]
            off += nb_e
        ib[I_EID[r]:I_EID[r] + NTILES * P] = _wrap16(eidx_flat).ravel()
        xb[X_DL[r]:X_DL[r] + P * NTILES] = np.ascontiguousarray(
            dl_flat.reshape(NTILES, P).T).astype(BF16).ravel()
    return {"xblob": xb, "iblob": ib, "xq8": q8}


def _prep_shared(inp):
    m = {}
    sD = 1.0 / math.sqrt(D)
    w = {}
    for t in ("A", "B"):
        w[f"kW_{t}"] = inp[f"kW_{t}"].reshape(4, P, C).astype(BF16)
        w[f"vWT_{t}"] = np.ascontiguousarray(inp[f"vW_{t}"].T).reshape(8, D, C).astype(BF16)
        w[f"qWT_{t}"] = np.ascontiguousarray(inp[f"qW_{t}"].T).reshape(8, D, C).astype(BF16)
        w[f"oW_{t}"] = inp[f"oW_{t}"].reshape(4, P, C).astype(BF16)
    w["linW"] = inp["linW"].reshape(4, P, 128).astype(BF16)
    for r, _, _ in RELS:
        w[f"mrel_{r}"] = np.ascontiguousarray(
            inp[f"mrel_{r}"].transpose(1, 0, 2)).reshape(D, C).astype(BF16)
        at = inp[f"arel_{r}"] * (inp[f"prel_{r}"] * sD)[:, None, None]
        w[f"arelT_{r}"] = np.ascontiguousarray(
            at.transpose(2, 0, 1)).reshape(D, C).astype(BF16)
    wflat = np.zeros(W_PAD, BF16)
    for nm, sh in W_MANIFEST:
        o = W_OFF[nm]
        wflat[o:o + int(np.prod(sh))] = w[nm].ravel()
    m["_wflat"] = wflat
    m["skp"] = np.array([[float(inp["skip_A"])], [float(inp["skip_B"])]], np.float32)
    for nm in ("kb_A", "kb_B", "ob_A", "ob_B"):
        m[nm] = np.asarray(inp[nm], np.float32).reshape(1, C)
    m["linb"] = np.asarray(inp["linb"], np.float32).reshape(1, 128)
    for t in ("A", "B"):
        for pfx in ("q", "v"):
            m[f"{pfx}b_{t}"] = np.ascontiguousarray(
                np.asarray(inp[f"{pfx}b_{t}"], np.float32).reshape(8, D).T)
    return m


# ---------------------------------------------------------------------------
# Device program
# ---------------------------------------------------------------------------

def _build(bz):
    nc = bacc.Bacc("TRN2", target_bir_lowering=False, debug=False,
                   enable_asserts=False, num_devices=NCORES)
    inp = {}

    def di(name, shape, dt):
        inp[name] = nc.dram_tensor(name, shape, dt, kind="ExternalInput").ap()

    di("xblob", [XB], bf)
    di("xq8", [Q8B], mybir.dt.uint8)
    di("iblob", [IB], i16)
    di("wshard", [WS], bf)
    di("skp", [2, 1], f32)
    for t in ("A", "B"):
        if not bz[f"kb_{t}"]:
            di(f"kb_{t}", [1, C], f32)
        if not bz[f"ob_{t}"]:
            di(f"ob_{t}", [1, C], f32)
        if not bz[f"qb_{t}"]:
            di(f"qb_{t}", [D, 8], f32)
        if not bz[f"vb_{t}"]:
            di(f"vb_{t}", [D, 8], f32)
    if not bz["linb"]:
        di("linb", [1, 128], f32)
    out = nc.dram_tensor("out", [XROWS, 128], bf, kind="ExternalOutput").ap()

    with tile.TileContext(nc) as tc:
        with ExitStack() as es:
            _program(es, tc, inp, out, bz)
    nc.compile()
    return nc


def _program(es, tc, inp, out, bz):
    nc = tc.nc
    wp = es.enter_context(tc.tile_pool(name="w", bufs=1))
    dp = es.enter_context(tc.tile_pool(name="d", bufs=1, space="DRAM"))
    sp = es.enter_context(tc.tile_pool(name="s", bufs=2))
    ep = es.enter_context(tc.tile_pool(name="e", bufs=2))
    gp = es.enter_context(tc.tile_pool(name="g", bufs=2))
    pp = es.enter_context(tc.tile_pool(name="p", bufs=3, space="PSUM"))
    agp = es.enter_context(tc.tile_pool(name="a", bufs=2, space="PSUM"))
    dnp = es.enter_context(tc.tile_pool(name="n", bufs=2, space="PSUM"))

    ident = wp.tile([P, P], bf, tag="ident", name="ident")
    make_identity(nc, ident[:])
    iota = wp.tile([P, TB, P], bf, tag="iota", name="iota")
    nc.sync.dma_start(
        iota[:], inp["xblob"][X_IOTA:X_IOTA + P * TB * P].rearrange(
            "(p t q) -> p t q", t=TB, q=P))

    # ---- decode packed e6m5 x into an internal bf16 table ----
    # bf16 bits = ((v & 0x7FF) + 3072) << 2 | (v >> 11) << 15,
    # with v = (hi & 0xFF) << 4 | nibble.
    u16 = mybir.dt.uint16
    u8 = mybir.dt.uint8
    xshd = dp.tile([XROWS, C], bf, tag="xshd", name="xshd")
    xsh = xshd[:]
    dcp = es.enter_context(tc.tile_pool(name="x8", bufs=1))
    KHI = XCHUNK * C          # hi-plane bytes per chunk
    GRP = XCHUNK // P         # decoded rows per partition (2)
    for ck in range(NXCH):
        hi8 = dcp.tile([P, KHI // P], u8, tag="hi8", name="hi8")
        nc.sync.dma_start(hi8[:], inp["xq8"][ck * KHI:(ck + 1) * KHI]
                          .rearrange("(p k) -> p k", p=P))
        nb8 = dcp.tile([P, KHI // (2 * P)], u8, tag="nb8", name="nb8")
        nc.sync.dma_start(nb8[:], inp["xq8"][Q8_HI + ck * KHI // 2:
                                             Q8_HI + (ck + 1) * KHI // 2]
                          .rearrange("(p k) -> p k", p=P))
        hiw = dcp.tile([P, KHI // P], u16, tag="hiw", name="hiw")
        nc.vector.tensor_copy(hiw[:], hi8[:])
        nbw = dcp.tile([P, KHI // (2 * P)], u16, tag="nbw", name="nbw")
        nc.vector.tensor_copy(nbw[:], nb8[:])
        ne = dcp.tile([P, KHI // (2 * P)], u16, tag="ne", name="ne")
        nc.vector.tensor_scalar(ne[:], nbw[:], 4, 0,
                                OP.logical_shift_right, OP.bitwise_or)
        nc.vector.tensor_scalar(nbw[:], nbw[:], 0xF, 0,
                                OP.bitwise_and, OP.bitwise_or)
        outw = dcp.tile([P, KHI // P], u16, tag="outw", name="outw")
        hv = hiw[:].rearrange("p (k two) -> p k two", two=2)
        ov = outw[:].rearrange("p (k two) -> p k two", two=2)

        def r3(ap):
            return ap.rearrange("p (k o) -> p k o", o=1)

        for half, nX in ((0, ne), (1, nbw)):
            t = dcp.tile([P, KHI // (2 * P)], u16, tag="t", name="t")
            nc.vector.tensor_scalar(r3(t[:]), hv[:, :, half:half + 1], 0x7F, 4,
                                    OP.bitwise_and, OP.logical_shift_left)
            nc.vector.tensor_tensor(t[:], t[:], nX[:], OP.bitwise_or)
            nc.vector.tensor_scalar(t[:], t[:], 3072, 0, OP.add, OP.add)
            nc.vector.tensor_scalar(t[:], t[:], 2, 0,
                                    OP.logical_shift_left, OP.bitwise_or)
            sg = dcp.tile([P, KHI // (2 * P)], u16, tag="sg", name="sg")
            nc.vector.tensor_scalar(r3(sg[:]), hv[:, :, half:half + 1], 7, 15,
                                    OP.logical_shift_right,
                                    OP.logical_shift_left)
            nc.vector.tensor_tensor(ov[:, :, half:half + 1], r3(t[:]),
                                    r3(sg[:]), OP.bitwise_or)
        nc.sync.dma_start(
            xsh[ck * XCHUNK:(ck + 1) * XCHUNK, :]
            .rearrange("(p g) c -> p (g c)", p=P, g=GRP),
            outw[:].bitcast(bf))

    # ---- all-gather x shards and weight shards ----
    xfull = dp.tile([XFULL, C], bf, tag="xfull", name="xfull",
                    addr_space="Shared")
    nc.gpsimd.collective_compute(
        "AllGather", OP.bypass,
        replica_groups=[list(range(NCORES))],
        ins=[xshd.opt()],
        outs=[xfull.opt()],
    )
    wint = dp.tile([1, WS], bf, tag="wint", name="wint")
    wfull = dp.tile([NCORES, WS], bf, tag="wfull", name="wfull",
                    addr_space="Shared")
    nc.gpsimd.dma_start(wint[:], inp["wshard"].rearrange("(o s) -> o s", o=1))
    nc.gpsimd.collective_compute(
        "AllGather", OP.bypass,
        replica_groups=[list(range(NCORES))],
        ins=[wint.opt()],
        outs=[wfull.opt()],
    )
    wflat = wfull[:].rearrange("o s -> (o s)")

    def load_w(name, shape, pat, **axes):
        t = wp.tile(list(shape), bf, tag=name, name=name)
        o = W_OFF[name]
        nc.sync.dma_start(
            t[:], wflat[o:o + int(np.prod(shape))].rearrange(pat, **axes))
        return t

    kW = {t: load_w(f"kW_{t}", (P, 4, C), "(c p o) -> p c o", c=4, p=P)
          for t in ("A", "B")}
    oW = {t: load_w(f"oW_{t}", (P, 4, C), "(c p o) -> p c o", c=4, p=P)
          for t in ("A", "B")}
    linW = load_w("linW", (P, 4, 128), "(c p o) -> p c o", c=4, p=P)
    mrel, arelT = {}, {}
    for r, _, _ in RELS:
        mrel[r] = load_w(f"mrel_{r}", (D, C), "(d c) -> d c", d=D)
        arelT[r] = load_w(f"arelT_{r}", (D, C), "(d c) -> d c", d=D)

    kb, ob, qb, vb, gate, gate1m = {}, {}, {}, {}, {}, {}
    for ti, t in enumerate(("A", "B")):
        for pfx, dd in (("kb", kb), ("ob", ob)):
            if not bz[f"{pfx}_{t}"]:
                row = wp.tile([1, C], f32, tag=f"{pfx}{t}r", name=f"{pfx}{t}r")
                nc.sync.dma_start(row[:], inp[f"{pfx}_{t}"])
                tt_ = wp.tile([P, C], f32, tag=f"{pfx}{t}", name=f"{pfx}{t}")
                nc.gpsimd.partition_broadcast(tt_[:], row[:])
                dd[t] = tt_
        for pfx, dd in (("qb", qb), ("vb", vb)):
            if not bz[f"{pfx}_{t}"]:
                tt_ = wp.tile([D, 8], f32, tag=f"{pfx}{t}", name=f"{pfx}{t}")
                nc.sync.dma_start(tt_[:], inp[f"{pfx}_{t}"])
                dd[t] = tt_
        sk = wp.tile([1, 1], f32, tag=f"sk{t}", name=f"sk{t}")
        nc.sync.dma_start(sk[:], inp["skp"][ti:ti + 1, :])
        g1_ = wp.tile([1, 1], f32, tag=f"g1r{t}", name=f"g1r{t}")
        nc.scalar.activation(g1_[:], sk[:], AF.Sigmoid)
        g_ = wp.tile([P, 1], f32, tag=f"g{t}", name=f"g{t}")
        nc.gpsimd.partition_broadcast(g_[:], g1_[:])
        gate[t] = g_
        g1 = wp.tile([P, 1], f32, tag=f"g1{t}", name=f"g1{t}")
        nc.vector.tensor_scalar(g1[:], g_[:], -1.0, 1.0, OP.mult, OP.add)
        gate1m[t] = g1
    linb = None
    if not bz["linb"]:
        lrow = wp.tile([1, 128], f32, tag="linbr", name="linbr")
        nc.sync.dma_start(lrow[:], inp["linb"])
        linb = wp.tile([P, 128], f32, tag="linb", name="linb")
        nc.gpsimd.partition_broadcast(linb[:], lrow[:])

    uidx_t, eidx_t, dl_t, qt_dram, kv_dram = {}, {}, {}, {}, {}
    for r, _, _ in RELS:
        ut_ = wp.tile([P, UCAP // 16], i16, tag=f"uidx{r}", name=f"uidx{r}")
        usrc_ap = inp["iblob"][I_UID[r]:I_UID[r] + UCAP].rearrange(
            "(p n) -> p n", p=16)
        for k in range(8):
            nc.sync.dma_start(ut_[16 * k:16 * (k + 1), :], usrc_ap)
        uidx_t[r] = ut_
        it_ = wp.tile([P, NTILES * 8], i16, tag=f"eidx{r}", name=f"eidx{r}")
        esrc_ap = inp["iblob"][I_EID[r]:I_EID[r] + NTILES * P].rearrange(
            "(p n) -> p n", p=16)
        for k in range(8):
            nc.sync.dma_start(it_[16 * k:16 * (k + 1), :], esrc_ap)
        eidx_t[r] = it_
        dt_ = wp.tile([P, NTILES], bf, tag=f"dl{r}", name=f"dl{r}")
        nc.sync.dma_start(
            dt_[:], inp["xblob"][X_DL[r]:X_DL[r] + P * NTILES].rearrange(
                "(p n) -> p n", n=NTILES))
        dl_t[r] = dt_
        qt_dram[r] = dp.tile([NSHP, C], bf, tag=f"qtd{r}", name=f"qtd{r}")
        kv_dram[r] = dp.tile([UCAP, 2 * C], bf, tag=f"kvd{r}", name=f"kvd{r}")

    # ---- stage 0: fuse relation transforms into projection weights ----
    # qWT/vWT are streamed from the gathered weight blob in [D, 8, P]
    # chunks instead of being kept resident (saves 32KB/partition SBUF).
    def wt_chunk_ap(name, cc):
        o = W_OFF[name]
        return wflat[o:o + 8 * D * C].rearrange(
            "(h p o) -> p h o", h=8, p=D)[:, :, cc * P:(cc + 1) * P]

    Wv, Wq = {}, {}
    for r, T, S in RELS:
        for nm, Wd, wt_name, rel_w in (("v", Wv, f"vWT_{S}", mrel[r]),
                                       ("q", Wq, f"qWT_{T}", arelT[r])):
            Wt = wp.tile([P, 4, C], bf, tag=f"W{nm}{r}", name=f"W{nm}{r}")
            for cc in range(4):
                wtc = wp.tile([D, 8, P], bf, tag="wtmp", name="wtmp", bufs=2)
                nc.sync.dma_start(wtc[:], wt_chunk_ap(wt_name, cc))
                ps = pp.tile([P, C], f32, tag="ps", name="ps")
                for h in range(H):
                    nc.tensor.matmul(
                        ps[:, h * D:(h + 1) * D],
                        wtc[:, h, :],
                        rel_w[:, h * D:(h + 1) * D],
                        start=True, stop=True)
                nc.scalar.copy(Wt[:, cc, :], ps[:])
            Wd[r] = Wt

    qbr, vbr = {}, {}
    for r, T, S in RELS:
        for dd, src_b, rel_w in ((vbr, vb.get(S), mrel[r]),
                                 (qbr, qb.get(T), arelT[r])):
            if src_b is None:
                continue
            ps = pp.tile([P, C], f32, tag="ps", name="ps")
            for h in range(H):
                nc.tensor.matmul(ps[:1, h * D:(h + 1) * D],
                                 src_b[:, h:h + 1],
                                 rel_w[:, h * D:(h + 1) * D],
                                 start=True, stop=True)
            sb_ = wp.tile([1, C], f32, tag=f"bs{r}{len(dd)}", name=f"bs{r}{len(dd)}")
            nc.vector.tensor_copy(sb_[:], ps[:1, :])
            rep = wp.tile([P, C], f32, tag=f"br{r}{len(dd)}", name=f"br{r}{len(dd)}")
            nc.gpsimd.partition_broadcast(rep[:], sb_[:])
            dd[r] = rep

    # ---- stage 1: q~ tables (own dst shard; reads xsh only) ----
    for r, T, S in RELS:
        tb = 0 if T == "A" else NSHP
        nt_list = [4] * (NSHP // 512) + ([1] if (NSHP % 512) else [])
        i0 = 0
        for ntile in nt_list:
            w = ntile * P
            xs = sp.tile([P, 4, 512], bf, tag="xq", name="xq")
            for cc in range(4):
                nc.sync.dma_start(
                    xs[:, cc, :w],
                    xsh[tb + i0: tb + i0 + w, cc * P:(cc + 1) * P],
                    transpose=True)
            qs = sp.tile([P, 4, C], bf, tag="qs", name="qs")
            for t in range(ntile):
                ps = pp.tile([P, C], f32, tag="ps", name="ps")
                for cc in range(4):
                    nc.tensor.matmul(ps[:], xs[:, cc, t * P:(t + 1) * P],
                                     Wq[r][:, cc, :], start=(cc == 0), stop=(cc == 3))
                if r in qbr:
                    nc.vector.tensor_tensor(qs[:, t, :], ps[:], qbr[r][:], OP.add)
                else:
                    nc.scalar.copy(qs[:, t, :], ps[:])
            nc.sync.dma_start(
                qt_dram[r][:][i0:i0 + w, :].rearrange("(g p) c -> p g c", p=P),
                qs[:, :ntile, :])
            i0 += w

    # ---- stage 2: kv tables (compact unique sources from xfull) ----
    for r, T, S in RELS:
        soff = 0 if S == "A" else NSHP
        for o in range(NCORES):
            obase = o * XROWS + soff
            for ck in range(UCHUNK):
                row0 = (o * UCHUNK + ck) * 512
                xs = sp.tile([P, 4, 512], bf, tag="xs2", name="xs2")
                nc.gpsimd.dma_gather(
                    xs[:], xfull[:][obase:obase + NSHP, :],
                    uidx_t[r][:, row0 // 16:(row0 + 512) // 16],
                    512, 512, 512, transpose=True)
                kvs = sp.tile([P, 4, 2 * C], bf, tag="kvs", name="kvs")
                for t in range(4):
                    psk = pp.tile([P, C], f32, tag="ps", name="ps")
                    for cc in range(4):
                        nc.tensor.matmul(psk[:], xs[:, cc, t * P:(t + 1) * P],
                                         kW[S][:, cc, :], start=(cc == 0), stop=(cc == 3))
                    if S in kb:
                        nc.vector.tensor_tensor(kvs[:, t, 0:C], psk[:], kb[S][:], OP.add)
                    else:
                        nc.scalar.copy(kvs[:, t, 0:C], psk[:])
                    psv = pp.tile([P, C], f32, tag="ps", name="ps")
                    for cc in range(4):
                        nc.tensor.matmul(psv[:], xs[:, cc, t * P:(t + 1) * P],
                                         Wv[r][:, cc, :], start=(cc == 0), stop=(cc == 3))
                    if r in vbr:
                        nc.vector.tensor_tensor(kvs[:, t, C:], psv[:], vbr[r][:], OP.add)
                    else:
                        nc.scalar.copy(kvs[:, t, C:], psv[:])
                nc.sync.dma_start(
                    kv_dram[r][:][row0:row0 + 512, :].rearrange("(g p) c -> p g c", p=P),
                    kvs[:])

    # ---- stage 3: edge phase + output, per dst block ----
    for T, rels in TYPE_RELS:
        toff = 0 if T == "A" else NSHP
        for blk in range(NBLK):
            xo = ep.tile([P, C], bf, tag="xo", name="xo")
            nc.sync.dma_start(xo[:], xsh[toff + blk * P: toff + (blk + 1) * P, :])
            norms = []
            for r in rels:
                kv = gp.tile([P, TB, 2 * C], bf, tag="kv", name="kv")
                nc.gpsimd.dma_gather(
                    kv[:], kv_dram[r][:],
                    eidx_t[r][:, blk * TB * 8:(blk + 1) * TB * 8],
                    TB * P, TB * P, 2 * C)
                qt = ep.tile([P, C], bf, tag="qt", name="qt")
                nc.sync.dma_start(qt[:], qt_dram[r][:][blk * P:(blk + 1) * P, :])
                agg = agp.tile([P, C], f32, tag="agg", name="agg")
                den = dnp.tile([P, H], f32, tag="den", name="den")
                mtT5 = ep.tile([P, TB, P], bf, tag="mtT", name="mtT")
                nc.vector.tensor_tensor(
                    mtT5[:], iota[:],
                    dl_t[r][:, blk * TB:(blk + 1) * TB]
                    .rearrange("p (t o) -> p t o", o=1).to_broadcast([P, TB, P]),
                    OP.is_equal)
                mps5 = pp.tile([P, TB, P], bf, tag="ps", name="ps")
                for t in range(TB):
                    nc.tensor.transpose(mps5[:, t, :], mtT5[:, t, :], ident[:])
                mt5 = ep.tile([P, TB, P], bf, tag="mt", name="mt")
                nc.scalar.copy(mt5[:], mps5[:])
                prod5 = ep.tile([P, TB, C], bf, tag="prod", name="prod")
                for t in range(TB):
                    qe = pp.tile([P, C], f32, tag="ps", name="ps")
                    nc.tensor.matmul(qe[:], mt5[:, t, :], qt[:], start=True, stop=True)
                    nc.vector.tensor_tensor(prod5[:, t, :], kv[:, t, 0:C], qe[:],
                                            OP.mult)
                L5 = ep.tile([P, TB * H], f32, tag="L", name="L")
                nc.vector.tensor_reduce(
                    L5[:].rearrange("p (t h) -> p t h", h=H),
                    prod5[:].rearrange("p t (h c) -> p t h c", h=H),
                    axis=mybir.AxisListType.X, op=OP.add)
                aT5 = ep.tile([P, TB * H], bf, tag="aT", name="aT")
                nc.scalar.activation(aT5[:], L5[:], AF.Exp)
                va5 = ep.tile([P, TB, C], bf, tag="va", name="va")
                nc.vector.tensor_tensor(
                    va5[:].rearrange("p t (h c) -> p t h c", h=H),
                    kv[:, :, C:].rearrange("p t (h c) -> p t h c", h=H),
                    aT5[:].rearrange("p (t h o) -> p t h o", o=1, h=H)
                    .to_broadcast([P, TB, H, D]),
                    OP.mult)
                for t in range(TB):
                    nc.tensor.matmul(agg[:], mtT5[:, t, :], va5[:, t, :],
                                     start=(t == 0), stop=(t == TB - 1))
                    nc.tensor.matmul(den[:], mtT5[:, t, :],
                                     aT5[:, t * H:(t + 1) * H],
                                     start=(t == 0), stop=(t == TB - 1))
                dn = ep.tile([P, H], f32, tag="dn", name="dn")
                nc.vector.tensor_scalar_add(dn[:], den[:], 1e-16)
                rec = ep.tile([P, H], f32, tag="rec", name="rec")
                nc.vector.reciprocal(rec[:], dn[:])
                nrm = ep.tile([P, C], f32 if len(rels) > 1 else bf,
                              tag=f"nrm{len(norms)}", name=f"nrm{len(norms)}")
                nc.vector.tensor_tensor(
                    nrm[:].rearrange("p (h c) -> p h c", h=H),
                    agg[:].rearrange("p (h c) -> p h c", h=H),
                    rec[:].rearrange("p (h o) -> p h o", o=1).to_broadcast([P, H, D]),
                    OP.mult)
                norms.append(nrm)
            if len(norms) > 1:
                gsum = ep.tile([P, C], bf, tag="gsum", name="gsum")
                nc.vector.tensor_tensor(gsum[:], norms[0][:], norms[1][:], OP.add)
            else:
                gsum = norms[0]
            gel = ep.tile([P, C], bf, tag="gel", name="gel")
            nc.scalar.activation(gel[:], gsum[:], AF.Gelu)
            gT = ep.tile([P, 4, P], bf, tag="gT", name="gT")
            tpg = pp.tile([P, 4, P], bf, tag="ps", name="ps")
            for cc in range(4):
                nc.tensor.transpose(tpg[:, cc, :], gel[:, cc * P:(cc + 1) * P], ident[:])
            nc.scalar.copy(gT[:], tpg[:])
            o_ps = pp.tile([P, C], f32, tag="ps", name="ps")
            for cc in range(4):
                nc.tensor.matmul(o_ps[:], gT[:, cc, :], oW[T][:, cc, :],
                                 start=(cc == 0), stop=(cc == 3))
            if T in ob:
                nc.vector.tensor_tensor(o_ps[:], o_ps[:], ob[T][:], OP.add)
            xg = ep.tile([P, C], f32, tag="xg", name="xg")
            nc.scalar.activation(xg[:], xo[:], AF.Copy, scale=gate1m[T][:])
            hb = ep.tile([P, C], bf, tag="hb", name="hb")
            nc.vector.scalar_tensor_tensor(hb[:], o_ps[:], gate[T][:], xg[:],
                                           OP.mult, OP.add)
            hT = ep.tile([P, 4, P], bf, tag="hT", name="hT")
            tph = pp.tile([P, 4, P], bf, tag="ps", name="ps")
            for cc in range(4):
                nc.tensor.transpose(tph[:, cc, :], hb[:, cc * P:(cc + 1) * P], ident[:])
            nc.scalar.copy(hT[:], tph[:])
            fin = pp.tile([P, 128], f32, tag="ps", name="ps")
            for cc in range(4):
                nc.tensor.matmul(fin[:], hT[:, cc, :], linW[:, cc, :],
                                 start=(cc == 0), stop=(cc == 3))
            fo = ep.tile([P, 128], bf, tag="fo", name="fo")
            if linb is not None:
                nc.vector.tensor_tensor(fo[:], fin[:], linb[:], OP.add)
            else:
                nc.scalar.copy(fo[:], fin[:])
            nc.sync.dma_start(out[toff + blk * P: toff + (blk + 1) * P, :], fo[:])


# ---------------------------------------------------------------------------
# Entry point
# ---------------------------------------------------------------------------

_CACHE = {}


def kernel(**inputs):
    inp = {k: np.asarray(v) for k, v in inputs.items()}
    shared = _prep_shared(inp)
    bz = {k: not np.any(np.asarray(inp[k])) for k in
          ("kb_A", "kb_B", "ob_A", "ob_B", "linb",
           "qb_A", "qb_B", "vb_A", "vb_B")}
    for k, z in bz.items():
        if z:
            shared.pop(k, None)
    wflat = shared.pop("_wflat")
    key = tuple(sorted(bz.items()))
    if key not in _CACHE:
        _CACHE[key] = _build(bz)
    nc = _CACHE[key]

    in_maps = []
    for core in range(NCORES):
        m = dict(shared)
        m["wshard"] = np.ascontiguousarray(wflat[core * WS:(core + 1) * WS])
        m.update(_prep_core(core, inp))
        in_maps.append(m)

    import time as _time
    _t0 = _time.time()
    res = run_bass_kernel_spmd(nc, in_maps, core_ids=list(range(NCORES)))
    kernel.last_run_s = _time.time() - _t0
    kernel.last_results = res

    full = np.zeros((2 * N, 128), np.float32)
    for core in range(NCORES):
        o = res.results[core]["out"].astype(np.float32)
        full[core * NSH:(core + 1) * NSH] = o[:NSH]
        full[N + core * NSH:N + (core + 1) * NSH] = o[NSHP:NSHP + NSH]
    return full
